# revision 17
# baseline (speedup 1.0000x reference)
"""Trainium2 Bass kernel for nn_Example1 (dense_transformer relation attention).

Reference math (b=32, n=1024, VOCAB=2048, D=3072):
    enc[b, j] = onehot(token[b, j], VOCAB) ++ onehot(j, n)          # 2 ones per row
    A = softmax_j(enc R enc^T + causal)
    logits = (A @ enc)[:, -1, :]

Only the LAST query row survives to the output, and enc is 2-hot, so the
whole computation collapses to (per sequence, t = token ids, tl = t[1023]):
    q       = R[tl, :] + R[3071, :]                       # row gather
    s[j]    = q[t_j] + q[2048 + j]                        # element gather
    A[j]    = softmax(s)[j]                               # last row unmasked
    out[2048 + j] = A[j]
    out[v]  = sum_{j: t_j == v} A[j]   for v < 2048        # weighted histogram

Device mapping (8 NeuronCores, data-parallel over batch, 4 sequences/core):
    - q rows:         GPSIMD indirect DMA row gathers from R in HBM
                      (table replicated per Q7 core for the element gather)
    - element gather: GPSIMD ap_gather from per-batch SBUF tables
    - softmax:        ScalarE exp with fused row-sum + DVE reciprocal
    - histogram:      one-hot decomposition 2048 = 64*32; TensorE matmuls
                      hist[a, c] = sum_j [t_j - (t_j&31) == 32a] * ([t_j&31 == c] * A_j)

kernel(**inputs) takes FULL inputs (token_ids [32, 1024] int, R [3072, 3072]
f32) and returns the FULL [32, 3072] f32 output. Host side only reshapes /
shards (layout marshalling of indices and iota/identity constants); all
data-dependent compute runs on device.
"""

from contextlib import ExitStack

import numpy as np

import concourse.bacc as bacc
import concourse.bass as bass
import concourse.mybir as mybir
import concourse.tile as tile
from concourse import library_config
from concourse.bass_utils import run_bass_kernel_spmd

VOCAB = 2048
CTX = 1024
D = VOCAB + CTX  # 3072
NCORES = 8
BPC = 4  # batches (sequences) per core

F32 = mybir.dt.float32
I32 = mybir.dt.int32
I16 = mybir.dt.int16
OP = mybir.AluOpType


def _emit(nc, tokw, tokc, tl4, tl128, iaf, icf, id4, R, out):
    """Per-core kernel body.

    tokw [128,32] i16: wrapped token idxs for ap_gather (core c=b+4h, batch b,
        j-half h; idx i of core c at [16c + i%16, i//16], value t[b, 512h+i]).
    tokc [128,32] i32: tokens with j on partitions; tokc[jj, 8b+k] = t[b, 128k+jj].
    tl4 [4,1], tl128 [128,1] i32: t[b, 1023] (tl128 row 16*(b+4h)+r = tl_b).
    iaf [128,2048] f32 const: iaf[p, 64*col + a] = 32*a.
    icf [128,1024] f32 const: icf[p, 32*col + c] = c.
    id4 [4,4] f32 const: identity.
    R [3072,3072] f32; out [4,3072] f32.
    """
    with tile.TileContext(nc) as tc, ExitStack() as ctx:
        pool = ctx.enter_context(tc.tile_pool(name="main", bufs=1))
        ppool = ctx.enter_context(tc.tile_pool(name="psum", bufs=2, space="PSUM"))
        hpool = ctx.enter_context(tc.tile_pool(name="hist", bufs=2, space="PSUM"))

        # ---- input loads ----
        tokw_s = pool.tile([128, 32], I16, name="tokw_s")
        nc.sync.dma_start(tokw_s[:], tokw)
        tokc_s = pool.tile([128, 32], I32, name="tokc_s")
        nc.sync.dma_start(tokc_s[:], tokc)
        ri = pool.tile([BPC, 1], I32, name="ri")
        nc.sync.dma_start(ri[:], tl4)
        idx128 = pool.tile([128, 1], I32, name="idx128")
        nc.sync.dma_start(idx128[:], tl128)
        iaf_s = pool.tile([128, VOCAB], F32, name="iaf_s")
        nc.sync.dma_start(iaf_s[:], iaf)
        icf_s = pool.tile([128, CTX], F32, name="icf_s")
        nc.sync.dma_start(icf_s[:], icf)
        id4_s = pool.tile([BPC, BPC], F32, name="id4_s")
        nc.sync.dma_start(id4_s[:], id4)
        r71b = pool.tile([128, VOCAB], F32, name="r71b")
        r71b_src = bass.AP(tensor=R.tensor, offset=3071 * D, ap=[[0, 128], [1, VOCAB]])
        nc.sync.dma_start(r71b[:], r71b_src)
        r71p = pool.tile([BPC, CTX], F32, name="r71p")
        r71p_src = bass.AP(tensor=R.tensor, offset=3071 * D + VOCAB,
                           ap=[[0, BPC], [1, CTX]])
        nc.sync.dma_start(r71p[:], r71p_src)

        # ---- indirect row gathers from R (SWDGE desc-gen on Pool, before the
        # library swap) ----
        tbl = pool.tile([128, VOCAB], F32, name="tbl")
        nc.gpsimd.indirect_dma_start(
            out=tbl[:], out_offset=None, in_=R,
            in_offset=bass.IndirectOffsetOnAxis(ap=idx128[:, 0:1], axis=0),
        )
        rtlp = pool.tile([BPC, CTX], F32, name="rtlp")
        nc.gpsimd.indirect_dma_start(
            out=rtlp[:], out_offset=None, in_=R,
            in_offset=bass.IndirectOffsetOnAxis(ap=ri[:, 0:1], axis=0),
            element_offset=VOCAB,
        )

        # library swap for ap_gather (~14us Pool-blocking; overlaps the DMAs
        # and DVE work below)
        nc.gpsimd.load_library(library_config.ap_gather)

        # ---- token-only one-hot pieces (off critical path) ----
        ci = pool.tile([128, 32], I32, name="ci")
        nc.vector.tensor_scalar(out=ci[:], in0=tokc_s[:], scalar1=31,
                                scalar2=None, op0=OP.bitwise_and)
        cf = pool.tile([128, 32], F32, name="cf")
        nc.vector.tensor_copy(cf[:], ci[:])
        df = pool.tile([128, 32], F32, name="df")  # 32*a = t - c, exact
        nc.vector.tensor_tensor(out=df[:], in0=tokc_s[:], in1=ci[:], op=OP.subtract)

        def bcast(src_tile, inner):
            # [128, 32] -> [128, 32, inner] view broadcasting along a new axis
            return bass.AP(tensor=src_tile[:].tensor, offset=0,
                           ap=[[32, 128], [1, 32], [0, inner]])

        one_a = pool.tile([128, VOCAB], F32, name="one_a")
        nc.vector.tensor_tensor(
            out=one_a[:].rearrange("p (c a) -> p c a", a=64),
            in0=iaf_s[:].rearrange("p (c a) -> p c a", a=64),
            in1=bcast(df, 64), op=OP.is_equal)
        one_c = pool.tile([128, CTX], F32, name="one_c")
        nc.vector.tensor_tensor(
            out=one_c[:].rearrange("p (c a) -> p c a", a=32),
            in0=icf_s[:].rearrange("p (c a) -> p c a", a=32),
            in1=bcast(cf, 32), op=OP.is_equal)

        # ---- q = R[tl] + R[3071] (vocab part replicated per core; pos part) ----
        nc.vector.tensor_tensor(out=tbl[:], in0=tbl[:], in1=r71b[:], op=OP.add)
        q4p = pool.tile([BPC, CTX], F32, name="q4p")
        nc.vector.tensor_tensor(out=q4p[:], in0=rtlp[:], in1=r71p[:], op=OP.add)

        # ---- element gather: core c=b+4h gathers q_b[t] for its 512 j ----
        gq = pool.tile([128, 512], F32, name="gq")
        nc.gpsimd.ap_gather(
            out_ap=gq[:].rearrange("c (i d) -> c i d", d=1),
            in_ap=tbl[:].rearrange("c (n d) -> c n d", d=1),
            idxs_ap=tokw_s[:],
            channels=128, num_elems=VOCAB, d=1, num_idxs=512,
        )

        # collect the 8 useful rows into [4, 1024] (DMA: engine ops cannot read
        # strided partitions at non-32-aligned bases)
        gqc = pool.tile([BPC, CTX], F32, name="gqc")
        nc.sync.dma_start(gqc[:, 0:512], gq[0:64:16, :])
        nc.sync.dma_start(gqc[:, 512:CTX], gq[64:128:16, :])

        # ---- scores + softmax (scores are tiny: skip max-subtraction) ----
        s4 = pool.tile([BPC, CTX], F32, name="s4")
        nc.vector.tensor_tensor(out=s4[:], in0=gqc[:], in1=q4p[:], op=OP.add)
        e4 = pool.tile([BPC, CTX], F32, name="e4")
        ssum = pool.tile([BPC, 1], F32, name="ssum")
        nc.scalar.activation(e4[:], s4[:], mybir.ActivationFunctionType.Exp,
                             accum_out=ssum[:])
        srec = pool.tile([BPC, 1], F32, name="srec")
        nc.vector.reciprocal(srec[:], ssum[:])
        a4 = pool.tile([BPC, CTX], F32, name="a4")
        nc.vector.tensor_scalar(out=a4[:], in0=e4[:], scalar1=srec[:, 0:1],
                                scalar2=None, op0=OP.mult)
        # positional half of the output
        nc.sync.dma_start(out[:, VOCAB:D], a4[:])

        # ---- transpose A to j-on-partitions: ac[jj, 8b+k] = A[b, 128k+jj] ----
        ac = pool.tile([128, 32], F32, name="ac")
        for k in range(8):
            tp = ppool.tile([128, BPC], F32, name="tp")
            nc.tensor.transpose(out=tp[:], in_=a4[:, 128 * k:128 * (k + 1)],
                                identity=id4_s[:])
            nc.scalar.copy(out=ac[:, k:32:8], in_=tp[:])

        # ---- hist[a, c] = sum_j oneA[j, a] * (oneC[j, c] * A_j) ----
        w_all = pool.tile([128, CTX], F32, name="w_all")
        nc.vector.tensor_tensor(
            out=w_all[:].rearrange("p (c a) -> p c a", a=32),
            in0=one_c[:].rearrange("p (c a) -> p c a", a=32),
            in1=bcast(ac, 32), op=OP.mult)

        for b in range(BPC):
            hp = hpool.tile([64, 32], F32, name="hp")
            for k in range(8):
                col = 8 * b + k
                nc.tensor.matmul(out=hp[:],
                                 lhsT=one_a[:, 64 * col:64 * (col + 1)],
                                 rhs=w_all[:, 32 * col:32 * (col + 1)],
                                 start=(k == 0), stop=(k == 7))
            hs = pool.tile([64, 32], F32, name=f"hs{b}")
            nc.scalar.copy(out=hs[:], in_=hp[:])
            dst = out[b:b + 1, 0:VOCAB].rearrange("one (a c) -> (one a) c", c=32)
            nc.sync.dma_start(dst, hs[:])


def build_nc():
    nc = bacc.Bacc("TRN2", target_bir_lowering=False, debug=False)
    tokw = nc.dram_tensor("tokw", [128, 32], I16, kind="ExternalInput")
    tokc = nc.dram_tensor("tokc", [128, 32], I32, kind="ExternalInput")
    tl4 = nc.dram_tensor("tl4", [BPC, 1], I32, kind="ExternalInput")
    tl128 = nc.dram_tensor("tl128", [128, 1], I32, kind="ExternalInput")
    iaf = nc.dram_tensor("iaf", [128, VOCAB], F32, kind="ExternalInput")
    icf = nc.dram_tensor("icf", [128, CTX], F32, kind="ExternalInput")
    id4 = nc.dram_tensor("id4", [BPC, BPC], F32, kind="ExternalInput")
    R = nc.dram_tensor("R", [D, D], F32, kind="ExternalInput")
    out = nc.dram_tensor("out", [BPC, D], F32, kind="ExternalOutput")
    _emit(nc, tokw.ap()[:, :], tokc.ap()[:, :], tl4.ap()[:, :],
          tl128.ap()[:, :], iaf.ap()[:, :], icf.ap()[:, :], id4.ap()[:, :],
          R.ap()[:, :], out.ap()[:, :])
    nc.compile()
    return nc


_NC_CACHE = None


def _get_nc():
    global _NC_CACHE
    if _NC_CACHE is None:
        _NC_CACHE = build_nc()
    return _NC_CACHE


def _consts():
    iaf = np.broadcast_to(
        (32 * np.arange(64, dtype=np.float32))[None, None, :],
        (128, 32, 64)).reshape(128, VOCAB)
    icf = np.broadcast_to(
        np.arange(32, dtype=np.float32)[None, None, :],
        (128, 32, 32)).reshape(128, CTX)
    id4 = np.eye(BPC, dtype=np.float32)
    return (np.ascontiguousarray(iaf), np.ascontiguousarray(icf), id4)


def _make_in_maps(token_ids, R):
    token_ids = np.asarray(token_ids).astype(np.int32)
    R = np.ascontiguousarray(np.asarray(R, dtype=np.float32))
    assert token_ids.shape == (NCORES * BPC, CTX), token_ids.shape
    assert R.shape == (D, D), R.shape
    iaf, icf, id4 = _consts()
    in_maps = []
    for c in range(NCORES):
        t = token_ids[c * BPC:(c + 1) * BPC]  # [4, 1024]
        # tokw[16*(b+4h)+r, s] = t[b, 512h+16s+r]  (ap_gather wrapped layout)
        tw = t.reshape(BPC, 2, 32, 16).transpose(1, 0, 3, 2).reshape(128, 32)
        # tokc[jj, 8b+k] = t[b, 128k+jj]
        tcc = t.reshape(BPC, 8, 128).transpose(2, 0, 1).reshape(128, 32)
        tl = t[:, -1].astype(np.int32)  # [4]
        tl128 = np.repeat(np.tile(tl, 2), 16).reshape(128, 1)
        in_maps.append({
            "tokw": np.ascontiguousarray(tw.astype(np.int16)),
            "tokc": np.ascontiguousarray(tcc.astype(np.int32)),
            "tl4": np.ascontiguousarray(tl.reshape(BPC, 1)),
            "tl128": np.ascontiguousarray(tl128),
            "iaf": iaf, "icf": icf, "id4": id4,
            "R": R,
        })
    return in_maps


def _run(token_ids, R, trace=False):
    nc = _get_nc()
    in_maps = _make_in_maps(token_ids, R)
    res = run_bass_kernel_spmd(nc, in_maps, list(range(NCORES)), trace=trace)
    full = np.concatenate([res.results[c]["out"] for c in range(NCORES)], axis=0)
    return full, res


def kernel(**inputs):
    token_ids = inputs["token_ids"]
    R = inputs["R"]
    full, _ = _run(token_ids, R, trace=False)
    return full


def kernel_profiled(**inputs):
    """Like kernel() but also returns the profiled HW exec time in ns."""
    full, res = _run(inputs["token_ids"], inputs["R"], trace=True)
    return full, res.exec_time_ns


# revision 20
# speedup vs baseline: 1.1603x; 1.1603x over previous
"""Trainium2 Bass kernel for nn_Example1 (dense_transformer relation attention).

Reference math (b=32, n=1024, VOCAB=2048, D=3072):
    enc[b, j] = onehot(token[b, j], VOCAB) ++ onehot(j, n)          # 2 ones per row
    A = softmax_j(enc R enc^T + causal)
    logits = (A @ enc)[:, -1, :]

Only the LAST query row survives to the output, and enc is 2-hot, so the
whole computation collapses to (per sequence, t = token ids, tl = t[1023]):
    q       = R[tl, :] + R[3071, :]                       # row gather
    s[j]    = q[t_j] + q[2048 + j]                        # element gather
    A[j]    = softmax(s)[j]                               # last row unmasked
    out[2048 + j] = A[j]
    out[v]  = sum_{j: t_j == v} A[j]   for v < 2048        # weighted histogram

Device mapping (8 NeuronCores, data-parallel over batch, 4 sequences/core):
    - q rows:         GPSIMD indirect DMA row gathers from R in HBM
                      (table replicated per Q7 core for the element gather)
    - element gather: GPSIMD ap_gather from per-batch SBUF tables
    - softmax:        ScalarE exp with fused row-sum + DVE reciprocal
    - histogram:      one-hot decomposition 2048 = 64*32; TensorE matmuls
                      hist[a, c] = sum_j [t_j - (t_j&31) == 32a] * ([t_j&31 == c] * A_j)

kernel(**inputs) takes FULL inputs (token_ids [32, 1024] int, R [3072, 3072]
f32) and returns the FULL [32, 3072] f32 output. Host side only reshapes /
shards (layout marshalling of indices and iota/identity constants); all
data-dependent compute runs on device.
"""

from contextlib import ExitStack

import numpy as np

import concourse.bacc as bacc
import concourse.bass as bass
import concourse.mybir as mybir
import concourse.tile as tile
from concourse import library_config
from concourse.bass_utils import run_bass_kernel_spmd

VOCAB = 2048
CTX = 1024
D = VOCAB + CTX  # 3072
NCORES = 8
BPC = 4  # batches (sequences) per core

F32 = mybir.dt.float32
I32 = mybir.dt.int32
I16 = mybir.dt.int16
OP = mybir.AluOpType


def _emit(nc, tokw, tokc, tl4, tl128, iaf, icf, id4, R, out):
    """Per-core kernel body.

    tokw [128,32] i16: wrapped token idxs for ap_gather (core c=b+4h, batch b,
        j-half h; idx i of core c at [16c + i%16, i//16], value t[b, 512h+i]).
    tokc [128,32] i32: tokens with j on partitions; tokc[jj, 8b+k] = t[b, 128k+jj].
    tl4 [4,1], tl128 [128,1] i32: t[b, 1023] (tl128 row 16*(b+4h)+r = tl_b).
    iaf [128,2048] f32 const: iaf[p, 64*col + a] = 32*a.
    icf [128,1024] f32 const: icf[p, 32*col + c] = c.
    id4 [4,4] f32 const: identity.
    R [3072,3072] f32; out [4,3072] f32.
    """
    with tile.TileContext(nc) as tc, ExitStack() as ctx:
        pool = ctx.enter_context(tc.tile_pool(name="main", bufs=1))
        ppool = ctx.enter_context(tc.tile_pool(name="psum", bufs=2, space="PSUM"))
        hpool = ctx.enter_context(tc.tile_pool(name="hist", bufs=2, space="PSUM"))

        # ---- input loads ----
        tokw_s = pool.tile([128, 32], I16, name="tokw_s")
        nc.sync.dma_start(tokw_s[:], tokw)
        tokc_s = pool.tile([128, 32], I32, name="tokc_s")
        nc.sync.dma_start(tokc_s[:], tokc)
        ri = pool.tile([BPC, 1], I32, name="ri")
        nc.sync.dma_start(ri[:], tl4)
        idx128 = pool.tile([128, 1], I32, name="idx128")
        nc.sync.dma_start(idx128[:], tl128)
        # big/less-critical loads go on the ACT HWDGE ring to overlap with the
        # SP-ring loads above
        iaf_s = pool.tile([128, VOCAB], F32, name="iaf_s")
        nc.scalar.dma_start(iaf_s[:], iaf)
        icf_s = pool.tile([128, CTX], F32, name="icf_s")
        nc.scalar.dma_start(icf_s[:], icf)
        id4_s = pool.tile([BPC, BPC], F32, name="id4_s")
        nc.scalar.dma_start(id4_s[:], id4)
        r71b = pool.tile([128, VOCAB], F32, name="r71b")
        r71b_src = bass.AP(tensor=R.tensor, offset=3071 * D, ap=[[0, 128], [1, VOCAB]])
        nc.scalar.dma_start(r71b[:], r71b_src)
        r71p = pool.tile([BPC, CTX], F32, name="r71p")
        r71p_src = bass.AP(tensor=R.tensor, offset=3071 * D + VOCAB,
                           ap=[[0, BPC], [1, CTX]])
        nc.scalar.dma_start(r71p[:], r71p_src)

        # ---- indirect row gathers from R (SWDGE desc-gen on Pool, before the
        # library swap) ----
        tbl = pool.tile([128, VOCAB], F32, name="tbl")
        nc.gpsimd.indirect_dma_start(
            out=tbl[:], out_offset=None, in_=R,
            in_offset=bass.IndirectOffsetOnAxis(ap=idx128[:, 0:1], axis=0),
        )
        rtlp = pool.tile([BPC, CTX], F32, name="rtlp")
        nc.gpsimd.indirect_dma_start(
            out=rtlp[:], out_offset=None, in_=R,
            in_offset=bass.IndirectOffsetOnAxis(ap=ri[:, 0:1], axis=0),
            element_offset=VOCAB,
        )

        # library swap for ap_gather (~15us Pool-blocking). Tile sinks the
        # swap barrier to just before the first ap_gather instruction, so fire
        # a tiny dummy gather on early-available tiles to overlap the IRAM
        # load with the table DMAs instead of serializing after them.
        nc.gpsimd.load_library(library_config.ap_gather)
        dummy = pool.tile([16, 16], F32, name="dummy")
        nc.gpsimd.ap_gather(
            out_ap=dummy[:].rearrange("c (i d) -> c i d", d=1),
            in_ap=iaf_s[0:16, :].rearrange("c (n d) -> c n d", d=1),
            idxs_ap=tokw_s[0:16, 0:1],
            channels=16, num_elems=VOCAB, d=1, num_idxs=16,
        )

        # ---- token-only one-hot pieces (off critical path) ----
        ci = pool.tile([128, 32], I32, name="ci")
        nc.vector.tensor_scalar(out=ci[:], in0=tokc_s[:], scalar1=31,
                                scalar2=None, op0=OP.bitwise_and)
        cf = pool.tile([128, 32], F32, name="cf")
        nc.vector.tensor_copy(cf[:], ci[:])
        df = pool.tile([128, 32], F32, name="df")  # 32*a = t - c, exact
        nc.vector.tensor_tensor(out=df[:], in0=tokc_s[:], in1=ci[:], op=OP.subtract)

        def bcast(src_tile, inner):
            # [128, 32] -> [128, 32, inner] view broadcasting along a new axis
            return bass.AP(tensor=src_tile[:].tensor, offset=0,
                           ap=[[32, 128], [1, 32], [0, inner]])

        one_a = pool.tile([128, VOCAB], F32, name="one_a")
        nc.vector.tensor_tensor(
            out=one_a[:].rearrange("p (c a) -> p c a", a=64),
            in0=iaf_s[:].rearrange("p (c a) -> p c a", a=64),
            in1=bcast(df, 64), op=OP.is_equal)
        one_c = pool.tile([128, CTX], F32, name="one_c")
        nc.vector.tensor_tensor(
            out=one_c[:].rearrange("p (c a) -> p c a", a=32),
            in0=icf_s[:].rearrange("p (c a) -> p c a", a=32),
            in1=bcast(cf, 32), op=OP.is_equal)

        # ---- q = R[tl] + R[3071] (vocab part replicated per core; pos part) ----
        nc.vector.tensor_tensor(out=tbl[:], in0=tbl[:], in1=r71b[:], op=OP.add)
        q4p = pool.tile([BPC, CTX], F32, name="q4p")
        nc.vector.tensor_tensor(out=q4p[:], in0=rtlp[:], in1=r71p[:], op=OP.add)

        # ---- element gather: core c=b+4h gathers q_b[t] for its 512 j ----
        gq = pool.tile([128, 512], F32, name="gq")
        nc.gpsimd.ap_gather(
            out_ap=gq[:].rearrange("c (i d) -> c i d", d=1),
            in_ap=tbl[:].rearrange("c (n d) -> c n d", d=1),
            idxs_ap=tokw_s[:],
            channels=128, num_elems=VOCAB, d=1, num_idxs=512,
        )

        # collect the 8 useful rows into [4, 1024] (DMA: engine ops cannot read
        # strided partitions at non-32-aligned bases)
        gqc = pool.tile([BPC, CTX], F32, name="gqc")
        nc.sync.dma_start(gqc[:, 0:512], gq[0:64:16, :])
        nc.sync.dma_start(gqc[:, 512:CTX], gq[64:128:16, :])

        # ---- scores + softmax (scores are tiny: skip max-subtraction) ----
        s4 = pool.tile([BPC, CTX], F32, name="s4")
        nc.vector.tensor_tensor(out=s4[:], in0=gqc[:], in1=q4p[:], op=OP.add)
        e4 = pool.tile([BPC, CTX], F32, name="e4")
        ssum = pool.tile([BPC, 1], F32, name="ssum")
        nc.scalar.activation(e4[:], s4[:], mybir.ActivationFunctionType.Exp,
                             accum_out=ssum[:])
        srec = pool.tile([BPC, 1], F32, name="srec")
        nc.vector.reciprocal(srec[:], ssum[:])
        a4 = pool.tile([BPC, CTX], F32, name="a4")
        nc.vector.tensor_scalar(out=a4[:], in0=e4[:], scalar1=srec[:, 0:1],
                                scalar2=None, op0=OP.mult)
        # positional half of the output
        nc.sync.dma_start(out[:, VOCAB:D], a4[:])

        # ---- transpose A to j-on-partitions: ac[jj, 8b+k] = A[b, 128k+jj] ----
        ac = pool.tile([128, 32], F32, name="ac")
        for k in range(8):
            tp = ppool.tile([128, BPC], F32, name="tp")
            nc.tensor.transpose(out=tp[:], in_=a4[:, 128 * k:128 * (k + 1)],
                                identity=id4_s[:])
            nc.scalar.copy(out=ac[:, k:32:8], in_=tp[:])

        # ---- hist[a, c] = sum_j oneA[j, a] * (oneC[j, c] * A_j) ----
        w_all = pool.tile([128, CTX], F32, name="w_all")
        nc.vector.tensor_tensor(
            out=w_all[:].rearrange("p (c a) -> p c a", a=32),
            in0=one_c[:].rearrange("p (c a) -> p c a", a=32),
            in1=bcast(ac, 32), op=OP.mult)

        hs = pool.tile([64, 4 * 32], F32, name="hs")
        for b in range(BPC):
            hp = hpool.tile([64, 32], F32, name="hp")
            for k in range(8):
                col = 8 * b + k
                nc.tensor.matmul(out=hp[:],
                                 lhsT=one_a[:, 64 * col:64 * (col + 1)],
                                 rhs=w_all[:, 32 * col:32 * (col + 1)],
                                 start=(k == 0), stop=(k == 7))
            nc.scalar.copy(out=hs[:, 32 * b:32 * (b + 1)], in_=hp[:])
        # one DMA for all four histograms: out[b, 32a + c] = hs[a, 32b + c]
        hist_dst = bass.AP(tensor=out.tensor, offset=0,
                           ap=[[32, 64], [D, BPC], [1, 32]])
        hist_src = bass.AP(tensor=hs[:].tensor, offset=0,
                           ap=[[128, 64], [32, BPC], [1, 32]])
        nc.sync.dma_start(hist_dst, hist_src)


def build_nc():
    nc = bacc.Bacc("TRN2", target_bir_lowering=False, debug=False)
    tokw = nc.dram_tensor("tokw", [128, 32], I16, kind="ExternalInput")
    tokc = nc.dram_tensor("tokc", [128, 32], I32, kind="ExternalInput")
    tl4 = nc.dram_tensor("tl4", [BPC, 1], I32, kind="ExternalInput")
    tl128 = nc.dram_tensor("tl128", [128, 1], I32, kind="ExternalInput")
    iaf = nc.dram_tensor("iaf", [128, VOCAB], F32, kind="ExternalInput")
    icf = nc.dram_tensor("icf", [128, CTX], F32, kind="ExternalInput")
    id4 = nc.dram_tensor("id4", [BPC, BPC], F32, kind="ExternalInput")
    R = nc.dram_tensor("R", [D, D], F32, kind="ExternalInput")
    out = nc.dram_tensor("out", [BPC, D], F32, kind="ExternalOutput")
    _emit(nc, tokw.ap()[:, :], tokc.ap()[:, :], tl4.ap()[:, :],
          tl128.ap()[:, :], iaf.ap()[:, :], icf.ap()[:, :], id4.ap()[:, :],
          R.ap()[:, :], out.ap()[:, :])
    nc.compile()
    return nc


_NC_CACHE = None


def _get_nc():
    global _NC_CACHE
    if _NC_CACHE is None:
        _NC_CACHE = build_nc()
    return _NC_CACHE


def _consts():
    iaf = np.broadcast_to(
        (32 * np.arange(64, dtype=np.float32))[None, None, :],
        (128, 32, 64)).reshape(128, VOCAB)
    icf = np.broadcast_to(
        np.arange(32, dtype=np.float32)[None, None, :],
        (128, 32, 32)).reshape(128, CTX)
    id4 = np.eye(BPC, dtype=np.float32)
    return (np.ascontiguousarray(iaf), np.ascontiguousarray(icf), id4)


def _make_in_maps(token_ids, R):
    token_ids = np.asarray(token_ids).astype(np.int32)
    R = np.ascontiguousarray(np.asarray(R, dtype=np.float32))
    assert token_ids.shape == (NCORES * BPC, CTX), token_ids.shape
    assert R.shape == (D, D), R.shape
    iaf, icf, id4 = _consts()
    in_maps = []
    for c in range(NCORES):
        t = token_ids[c * BPC:(c + 1) * BPC]  # [4, 1024]
        # tokw[16*(b+4h)+r, s] = t[b, 512h+16s+r]  (ap_gather wrapped layout)
        tw = t.reshape(BPC, 2, 32, 16).transpose(1, 0, 3, 2).reshape(128, 32)
        # tokc[jj, 8b+k] = t[b, 128k+jj]
        tcc = t.reshape(BPC, 8, 128).transpose(2, 0, 1).reshape(128, 32)
        tl = t[:, -1].astype(np.int32)  # [4]
        tl128 = np.repeat(np.tile(tl, 2), 16).reshape(128, 1)
        in_maps.append({
            "tokw": np.ascontiguousarray(tw.astype(np.int16)),
            "tokc": np.ascontiguousarray(tcc.astype(np.int32)),
            "tl4": np.ascontiguousarray(tl.reshape(BPC, 1)),
            "tl128": np.ascontiguousarray(tl128),
            "iaf": iaf, "icf": icf, "id4": id4,
            "R": R,
        })
    return in_maps


def _run(token_ids, R, trace=False):
    nc = _get_nc()
    in_maps = _make_in_maps(token_ids, R)
    res = run_bass_kernel_spmd(nc, in_maps, list(range(NCORES)), trace=trace)
    full = np.concatenate([res.results[c]["out"] for c in range(NCORES)], axis=0)
    return full, res


def kernel(**inputs):
    token_ids = inputs["token_ids"]
    R = inputs["R"]
    full, _ = _run(token_ids, R, trace=False)
    return full


def kernel_profiled(**inputs):
    """Like kernel() but also returns the profiled HW exec time in ns."""
    full, res = _run(inputs["token_ids"], inputs["R"], trace=True)
    return full, res.exec_time_ns
